# revision 1
# baseline (speedup 1.0000x reference)
"""PSENet-style OHEM + dice loss on 8 Trainium2 NeuronCores.

Data-parallel over the batch: core b processes image b entirely on-chip
(one pass over its 24.5 MB of inputs; the kernel is HBM-bandwidth bound).
Each core emits 22 partial sums; the final scalar means are combined on
the host (they are 8x22 floats - no collective needed).

Math notes (labels / masks are exactly 0.0/1.0):
  dice(x, g, M) needs  a = sum(sig(x)*g*M), b = sum(sig(x)^2*M), c = sum(g*M).
  - masked logits: xm = min(x, (2M-1)*BIG)  =>  sig(xm) = sig(x)*M (up to
    sig(-BIG) ~ 2e-22, far below fp32 noise on these sums). One DVE
    scalar_tensor_tensor (STT) pass.
  - a: DVE STT (g*1.0)*sig with accum_out - product + sum in one pass.
  - b: ACT Square with accum_out (sum of squares) in one pass.
  - c: sum(g*M): GPSIMD tensor_mul product + ACT Copy/accum reduce for the
    six kernel channels (keeps DVE free); one DVE STT+accum for the text
    channel. (tensor_tensor_reduce would do it in one DVE pass but crashes
    the device - NRT_EXEC_UNIT_UNRECOVERABLE - so it is avoided.)
  - accum_out columns land in [128,16] per-engine accumulators; one
    [128,16]x[128,1] ones-matmul per accumulator does the cross-partition
    reduction; host combines 8x32 floats into the final three scalars.
  - The last kernel channel is processed as two half-tiles so the compute
    tail after the final DMA is halved (single-shot latency).
  - OHEM: for these inputs 3*pos_num >= total_neg for every image, so the
    OHEM threshold is the minimum negative score and the selected mask is
    exactly the training mask. The host VERIFIES the sufficient condition
    (RATIO+1)*pos_num >= N (pos_num = text-channel c; since sum_g >= pos_num
    this implies RATIO*pos_num >= total_neg, ~28 sigma of margin here) and
    falls back to a full host reference if it ever failed.

Single-shot latency tuning (cost-model trace verified): x6/m/g6 DMAs are
issued first and the text xm+sigmoid run before the M/Mb mask builds, so
ACT starts ~4 us earlier; x/g pools use 3 buffers so the DMA stream never
stalls on the head compute chain (was an 8.7 us gap); the split last
channel keeps the post-final-DMA tail short.

Measured on 8 axon-tunneled trn2 cores: steady-state ~62 us/image at the
~24.5 MB / ~400 GB/s-per-core HBM roofline (DMA busy 68 us modeled with
zero mid-stream gaps; engines below it: ACT ~63, DVE ~61, GPSIMD ~33 us).
Cost-model single-shot estimate 88 us (was 95.6 before the reorder).
"""

import os
import sys

import numpy as np

for _p in ("/opt/trn_rl_repo", "/root/.axon_site/_ro/trn_rl_repo"):
    if os.path.isdir(_p) and _p not in sys.path:
        sys.path.append(_p)

import concourse.bacc as bacc
import concourse.tile as tile
from concourse import mybir
from concourse.bass_utils import run_bass_kernel_spmd

B, C, H, W = 8, 7, 640, 640
NK = C - 1            # kernel channels
N = H * W             # pixels per image
P = 128               # SBUF partitions
F = N // P            # free dim per tile (3200)
BIG = 50.0
NCORES = 8
LAMBDA = 0.7
RATIO = 3

_dt = mybir.dt.float32
_AF = mybir.ActivationFunctionType
_ALU = mybir.AluOpType


def _img_ap(dram_ap):
    """[H, W] dram slab -> [128, 3200] partition-major access pattern."""
    return dram_ap.rearrange("(p q) w -> p (q w)", p=P)


def build_nc(debug=False, reps=1):
    nc = bacc.Bacc("TRN2", target_bir_lowering=False, debug=debug)
    x_d = nc.dram_tensor("x", [C, H, W], _dt, kind="ExternalInput")
    g_d = nc.dram_tensor("g", [C, H, W], _dt, kind="ExternalInput")
    m_d = nc.dram_tensor("m", [H, W], _dt, kind="ExternalInput")
    res_d = nc.dram_tensor("res", [16, 2], _dt, kind="ExternalOutput")

    with (
        tile.TileContext(nc) as tc,
        tc.tile_pool(name="const", bufs=1) as cpool,
        tc.tile_pool(name="xin", bufs=3) as xpool,
        tc.tile_pool(name="gin", bufs=3) as gpool,
        tc.tile_pool(name="xmp", bufs=2) as xmpool,
        tc.tile_pool(name="sbp", bufs=2) as sbpool,
        tc.tile_pool(name="junk", bufs=2) as jpool,
        tc.tile_pool(name="ps", bufs=1, space="PSUM") as ppool,
    ):
        # accum_out columns; res col 0 = partition-sums of acc_dve,
        # res col 1 = partition-sums of acc_act (see _combine for layout)
        acc_dve = cpool.tile([P, 16], _dt)
        acc_act = cpool.tile([P, 16], _dt)
        ones_t = cpool.tile([P, 1], _dt)
        nc.gpsimd.memset(ones_t[:], 1.0)
        nc.vector.memset(acc_dve[:], 0.0)
        nc.scalar.memzero(acc_act[:])

        def image_body(rep):
            # ---- text channel (ch 6) first: its logits gate everything ----
            xt6 = xpool.tile([P, F], _dt, tag="xin", name=f"xt6_r{rep}")
            nc.sync.dma_start(xt6[:], _img_ap(x_d.ap()[C - 1]))
            m_t = cpool.tile([P, F], _dt, tag="m_t", name=f"m_t_r{rep}")
            nc.sync.dma_start(m_t[:], _img_ap(m_d.ap()))
            gt6 = gpool.tile([P, F], _dt, tag="gin", name=f"gt6_r{rep}")
            nc.sync.dma_start(gt6[:], _img_ap(g_d.ap()[C - 1]))


            # kernel-channel mask M = (x_text > 0) * m, and (2M-1)*BIG.
            # Emitted after the text xm/sigma (see channel_body call below)
            # would be ideal, but M/Mb only gate kernel channels; keep them
            # after mb so the text sigmoid starts as early as possible.
            M_t = cpool.tile([P, F], _dt, tag="M_t", name=f"M_t_r{rep}")
            Mb_t = cpool.tile([P, F], _dt, tag="Mb_t", name=f"Mb_t_r{rep}")


            def channel_body(xt, gt, maskb, msel, col):
                # xm = min(x, maskb)   (DVE)
                xm = xmpool.tile([P, F], _dt, tag="xmp", name=f"xm{col}_r{rep}")
                nc.vector.scalar_tensor_tensor(
                    xm[:], xt[:], 0.0, maskb[:], _ALU.add, _ALU.min
                )
                # sb = sigmoid(xm)     (ACT)
                sbt = sbpool.tile([P, F], _dt, tag="sbp", name=f"sb{col}_r{rep}")
                nc.scalar.activation(sbt[:], xm[:], _AF.Sigmoid)
                # b = sum(sb^2)        (ACT, overwrite xm with junk output)
                nc.scalar.activation(
                    xm[:], sbt[:], _AF.Square,
                    accum_out=acc_act[:, col:col + 1],
                )
                # a = sum(g * sb)      (DVE STT w/ accum, overwrite xt)
                nc.vector.scalar_tensor_tensor(
                    xt[:], gt[:], 1.0, sbt[:], _ALU.mult, _ALU.mult,
                    accum_out=acc_dve[:, col:col + 1],
                )
                # c = sum(g * msel): GPSIMD product + ACT Copy/accum reduce
                jg = jpool.tile([P, F], _dt, tag="junk", name=f"jg{col}_r{rep}")
                nc.gpsimd.tensor_mul(jg[:], gt[:], msel[:])
                nc.scalar.activation(
                    xm[:], jg[:], _AF.Copy,
                    accum_out=acc_act[:, 8 + col:9 + col],
                )


            mb_t = cpool.tile([P, F], _dt, tag="mb_t", name=f"mb_t_r{rep}")
            nc.vector.tensor_scalar(
                mb_t[:], m_t[:], 2.0 * BIG, -BIG, _ALU.mult, _ALU.add
            )
            # text xm + sigma first (only needs x6 + mb); then M/Mb
            xm6 = xmpool.tile([P, F], _dt, tag="xmp", name=f"xm6_r{rep}")
            sb6 = sbpool.tile([P, F], _dt, tag="sbp", name=f"sb6_r{rep}")
            nc.vector.scalar_tensor_tensor(
                xm6[:], xt6[:], 0.0, mb_t[:], _ALU.add, _ALU.min
            )
            nc.scalar.activation(sb6[:], xm6[:], _AF.Sigmoid)
            nc.vector.scalar_tensor_tensor(
                M_t[:], xt6[:], 0.0, m_t[:], _ALU.is_gt, _ALU.mult
            )
            nc.vector.tensor_scalar(
                Mb_t[:], M_t[:], 2.0 * BIG, -BIG, _ALU.mult, _ALU.add
            )
            nc.scalar.activation(
                xm6[:], sb6[:], _AF.Square,
                accum_out=acc_act[:, 0:1]
            )
            nc.vector.scalar_tensor_tensor(
                xt6[:], gt6[:], 1.0, sb6[:], _ALU.mult, _ALU.mult,
                accum_out=acc_dve[:, 0:1],
            )
            nc.vector.scalar_tensor_tensor(
                xt6[:], gt6[:], 1.0, m_t[:], _ALU.mult, _ALU.mult,
                accum_out=acc_dve[:, 8:9],
            )


            for k in range(NK - 1):
                xt = xpool.tile([P, F], _dt, tag="xin", name=f"xk{k}_r{rep}")
                nc.sync.dma_start(xt[:], _img_ap(x_d.ap()[k]))
                gt = gpool.tile([P, F], _dt, tag="gin", name=f"gk{k}_r{rep}")
                nc.sync.dma_start(gt[:], _img_ap(g_d.ap()[k]))
                channel_body(xt, gt, Mb_t, M_t, 1 + k)

            # last kernel channel in two half-tiles so the post-final-DMA
            # compute tail is half as long (single-shot latency)
            k = NK - 1
            Fh = F // 2
            xt = xpool.tile([P, F], _dt, tag="xin", name=f"xk{k}_r{rep}")
            gt = gpool.tile([P, F], _dt, tag="gin", name=f"gk{k}_r{rep}")
            for h, (acol, bcol, ccol) in enumerate(((NK, NK, 14), (7, 8, 15))):
                xs = xt[:, h * Fh:(h + 1) * Fh]
                gs = gt[:, h * Fh:(h + 1) * Fh]
                src = _img_ap(x_d.ap()[k])
                nc.sync.dma_start(xs, src[:, h * Fh:(h + 1) * Fh])
                srcg = _img_ap(g_d.ap()[k])
                nc.sync.dma_start(gs, srcg[:, h * Fh:(h + 1) * Fh])
                xm = xmpool.tile([P, F], _dt, tag="xmp", name=f"xmL{h}_r{rep}")
                nc.vector.scalar_tensor_tensor(
                    xm[:, :Fh], xs, 0.0, Mb_t[:, h * Fh:(h + 1) * Fh],
                    _ALU.add, _ALU.min
                )
                sbt = sbpool.tile([P, F], _dt, tag="sbp", name=f"sbL{h}_r{rep}")
                nc.scalar.activation(sbt[:, :Fh], xm[:, :Fh], _AF.Sigmoid)
                nc.scalar.activation(
                    xm[:, :Fh], sbt[:, :Fh], _AF.Square,
                    accum_out=acc_act[:, bcol:bcol + 1],
                )
                nc.vector.scalar_tensor_tensor(
                    xm[:, Fh:2 * Fh], gs, 1.0, sbt[:, :Fh],
                    _ALU.mult, _ALU.mult,
                    accum_out=acc_dve[:, acol:acol + 1],
                )
                nc.vector.scalar_tensor_tensor(
                    sbt[:, Fh:2 * Fh], gs, 1.0,
                    M_t[:, h * Fh:(h + 1) * Fh], _ALU.mult, _ALU.mult,
                    accum_out=acc_dve[:, ccol:ccol + 1],
                )

        for rep in range(reps):
            image_body(rep)

        # cross-partition reduction of all accumulators with one ones-vector
        # matmul per accumulator: res row i <- sum_p acc[p, i]
        pr = ppool.tile([16, 2], _dt, tag="pr")
        nc.tensor.matmul(pr[:, 0:1], lhsT=acc_dve[:], rhs=ones_t[:],
                         start=True, stop=True)
        nc.tensor.matmul(pr[:, 1:2], lhsT=acc_act[:], rhs=ones_t[:],
                         start=True, stop=True)
        res_sb = cpool.tile([16, 2], _dt)
        nc.scalar.copy(res_sb[:], pr[:])
        nc.sync.dma_start(res_d.ap(), res_sb[:])

    nc.compile()
    return nc


_CACHE = {}


def _get_nc():
    if "nc" not in _CACHE:
        _CACHE["nc"] = build_nc(debug=False)
    return _CACHE["nc"]


def _combine(res_list):
    """res_list: per-image [16, 2] device sums -> (loss_text, loss_kernels, loss).

    Returns None if the OHEM fast-path precondition fails for any image.
    """
    lt_b = np.zeros(B, np.float64)
    lk_b = np.zeros(B, np.float64)
    for b in range(B):
        v = np.asarray(res_list[b], np.float64)
        a_t, b_t = v[0, 0], v[0, 1]
        c_t = v[8, 0]            # text c is the DVE accumulator slot
        pos_num = c_t                    # sum(gt_text * m), exact integer
        # sel == m iff pos_num == 0 (fallback) or neg_num == total_neg,
        # i.e. RATIO*pos_num >= total_neg = N - sum_g. Since sum_g >=
        # sum_g*m = pos_num, (RATIO+1)*pos_num >= N is sufficient and
        # avoids computing sum_g on device (~28 sigma of margin here).
        if not (pos_num == 0 or (RATIO + 1) * pos_num >= N):
            return None
        lt_b[b] = 1.0 - 2.0 * a_t / (b_t + 0.001 + c_t + 0.001)
        lk = 0.0
        for k in range(NK):
            a_k, b_k = v[1 + k, 0], v[1 + k, 1]
            c_k = v[9 + k, 0] + v[9 + k, 1]
            if k == NK - 1:  # second half of the split last channel
                a_k += v[7, 0]
                b_k += v[8, 1]
                c_k = v[14, 0] + v[15, 0]
            lk += 1.0 - 2.0 * a_k / (b_k + 0.001 + c_k + 0.001)
        lk_b[b] = lk / NK
    lt = np.float32(lt_b.mean())
    lk = np.float32(lk_b.mean())
    loss = np.float32(LAMBDA) * lt + np.float32(1.0 - LAMBDA) * lk
    return (lt, lk, np.float32(loss))


def _numpy_reference(outputs, labels, training_masks):
    """Full-fidelity host fallback (mirrors the original loss exactly)."""
    def sigmoid(z):
        return 1.0 / (1.0 + np.exp(-z, dtype=np.float64))

    texts = outputs[:, -1].reshape(B, N).astype(np.float64)
    kernels = outputs[:, :-1].reshape(B, NK, N).astype(np.float64)
    gt_texts = labels[:, -1].reshape(B, N).astype(np.float64)
    gt_kernels = labels[:, :-1].reshape(B, NK, N).astype(np.float64)
    tm = training_masks.reshape(B, N).astype(np.float64)

    pos = gt_texts > 0.5
    pos_num = np.sum(pos & (tm > 0.5), axis=1)
    neg = ~pos
    total_neg = np.sum(neg, axis=1)
    neg_num = np.minimum(pos_num * RATIO, total_neg)
    neg_scores = np.where(neg, texts, -np.inf)
    sorted_desc = -np.sort(-neg_scores, axis=1)
    idx = np.clip(neg_num - 1, 0, N - 1)
    thr = np.take_along_axis(sorted_desc, idx[:, None], axis=1)
    sel = (((texts >= thr) | pos) & (tm > 0.5)).astype(np.float64)
    fallback = (pos_num == 0) | (neg_num == 0)
    sel = np.where(fallback[:, None], tm, sel)

    def dice(inp, target, mask):
        p = sigmoid(inp) * mask
        t = target * mask
        a = np.sum(p * t, axis=-1)
        bb = np.sum(p * p, axis=-1) + 0.001
        cc = np.sum(t * t, axis=-1) + 0.001
        return 1.0 - 2.0 * a / (bb + cc)

    loss_text = dice(texts, gt_texts, sel).mean()
    sel_k = ((sigmoid(texts) > 0.5) & (tm > 0.5)).astype(np.float64)
    loss_kernels = dice(kernels, gt_kernels, sel_k[:, None, :]).mean(axis=1).mean()
    loss = LAMBDA * loss_text + (1.0 - LAMBDA) * loss_kernels
    return (np.float32(loss_text), np.float32(loss_kernels), np.float32(loss))


def kernel(outputs, labels, training_masks):
    outputs = np.asarray(outputs, dtype=np.float32)
    labels = np.asarray(labels, dtype=np.float32)
    training_masks = np.asarray(training_masks, dtype=np.float32)
    assert outputs.shape == (B, C, H, W)

    nc = _get_nc()
    in_maps = [
        {
            "x": np.ascontiguousarray(outputs[b]),
            "g": np.ascontiguousarray(labels[b]),
            "m": np.ascontiguousarray(training_masks[b]),
        }
        for b in range(B)
    ]
    r = None
    for attempt in range(3):
        try:
            r = run_bass_kernel_spmd(
                nc, in_maps, list(range(NCORES)),
                trace=_CACHE.get("trace", False),
            )
            break
        except Exception:
            if attempt == 2:
                raise
            _CACHE.pop("nc", None)
            nc = _get_nc()
    _CACHE["last_result"] = r
    res_list = [r.results[b]["res"] for b in range(B)]
    out = _combine(res_list)
    if out is None:
        # OHEM threshold is not the minimum negative score -> exact host path
        out = _numpy_reference(outputs, labels, training_masks)
    return out

